# revision 54
# baseline (speedup 1.0000x reference)
"""MoE routed matmul kernel for Trainium2 (8 NeuronCores, expert-parallel).

Problem: out[b, u] = sum_d x[b, d] * embeddings[content_idx[b], d, u]
with B=256 examples, D=U=1024, C=64 experts (256 MB fp32 table).

Strategy (expert parallel, fp8 weights):
  - Core k owns 8 expert slots. It streams its expert matrices from
    HBM once. Weights ride as fp8 e3m4 (1 byte/elem) with per-column
    scales applied on the host after gather — 8.4 MB/core, the memory
    roofline, vs 33.5 MB for the bf16 hi/lo baseline. The two HWDGE
    rings (SP+ACT, ~180 GB/s each) stream 512 KB chunks and hold the
    whole 8-expert working set in SBUF so the rings free-run ahead of
    the PE (finer chunking starves the rings: each DMA trigger
    instruction costs ~0.6 us on the issuing engine).
  - Orientation is flipped vs the baseline: W is the STATIONARY
    operand in [128, 128] blocks (full-width LDWEIGHTS → the backend
    enables FWL) and the grouped x is the MOVING operand. Per expert
    that is 64 LDWEIGHTS + 64 short matmuls (2*cap rows each); the
    LDW+MM pair sustains ~64 cycles, so the PE tracks the DMA stream
    with headroom once it p-state-ramps to 2.4 GHz.
  - x rides as fp8 hi/lo (x*g = qxh + qxl/32, each e3m4) stacked
    along the moving free dim; a 2-op DVE fold (hi + lo/32) runs per
    expert, overlapped with the stream. e3m4 products are exact in
    the PE datapath, so device numerics match the host simulation;
    end-to-end rel err ~1.3e-2 is set by the fp8 weight rounding,
    comfortably under the 2e-2 gate.
  - Unused experts are permuted into the early slots of core 0 and
    their weight DMAs are predicated off via a runtime mask
    (dma_start cond=): the matmuls run on stale SBUF at the stream
    head where the PE would idle anyway, and the host discards those
    outputs. Outputs stay resident in SBUF (op pool holds all 8) so
    the slow SWDGE out-path never back-pressures the PE.
  - Host: groups examples by expert into cap padded slots, lays W/x
    out in the exact SBUF images the PE wants, and after gather
    applies the per-column weight scales and 1/g (host pre/post is
    index bookkeeping + O(B*U) scaling; all O(B*D*U) math is on
    device).
"""

import numpy as np
import ml_dtypes

from concourse import bacc, mybir, tile
from concourse import bass_utils

E3M4 = ml_dtypes.float8_e3m4

B, D, U, C = 256, 1024, 1024, 64
NCORES = 8
EPC = C // NCORES          # experts per core
KC = D // 128              # 128-deep k-chunks per expert
NJ = U // 128              # 128-wide u-blocks per expert
NBP = 2                    # DMA chunks per expert (4 u-blocks each)

_compiled = {}


def _build_fp8(cap: int):
    """Per-core SPMD program: fp8 e3m4 weights stationary, x moving."""
    f32 = mybir.dt.float32
    fp8 = mybir.dt.float8e3
    cap2 = 2 * cap
    nc = bacc.Bacc("TRN2", target_bir_lowering=False, debug=False)
    whl = nc.dram_tensor("whl", [EPC, 128, KC * NJ * 128], fp8,
                         kind="ExternalInput").ap()
    xt = nc.dram_tensor("xt", [128, EPC * KC * cap2], fp8,
                        kind="ExternalInput").ap()
    em = nc.dram_tensor("emask", [1, EPC], mybir.dt.int32,
                        kind="ExternalInput").ap()
    out = nc.dram_tensor("out", [EPC, 128, NJ * cap], f32,
                         kind="ExternalOutput").ap()
    trig_engines = (mybir.EngineType.SP, mybir.EngineType.Activation)

    # per-partition PSUM is 16 KB; a tile is NJ*cap2 fp32 bytes
    psum_bufs = max(1, min(4, 16384 // (NJ * cap2 * 4)))
    with tile.TileContext(nc) as tc:
        with tc.tile_pool(name="wp", bufs=NBP * EPC) as wp, \
             tc.tile_pool(name="xp", bufs=1) as xp, \
             tc.tile_pool(name="pp", bufs=psum_bufs, space="PSUM") as pp, \
             tc.tile_pool(name="tp", bufs=4) as tp, \
             tc.tile_pool(name="op", bufs=EPC + 1) as op, \
             tc.tile_pool(name="mp", bufs=1) as mp:
            xt_t = xp.tile([128, EPC * KC * cap2], fp8)
            # expert-used mask rides the sync ring head (32 B, lands
            # ~instantly; SWDGE's first-DMA latency is ~3.5 us — too
            # late) so the batched predicate load completes before the
            # first conditional trigger.
            em_t = mp.tile([1, EPC], mybir.dt.int32)
            nc.sync.dma_start(em_t[:], em[:])
            held = []
            conds = []
            for e in range(EPC):
                # Whole expert = [128, 8 KB] fp8, contiguous per
                # partition; 512 KB chunks across both HWDGE rings
                # (trigger instructions cost ~0.6 us each, so finer
                # chunking starves the rings); the last two experts use
                # fine chunks for a short warm drain. The wp pool holds
                # the full stream so the rings free-run ahead of the PE.
                # Unused experts (host routes them to early slots) skip
                # their weight DMAs via the runtime mask; their matmuls
                # run on stale SBUF and the host discards the outputs.
                # The first two slots are unconditional runway; the six
                # predicates load in one batch at slot 2 (per-expert
                # loads pace triggers at ~2-3 us apart).
                if e == 2:
                    conds.extend(nc.values_load_multi_w_load_instructions(
                        em_t[0:1, 2:EPC], engines=trig_engines,
                        min_val=0, max_val=1,
                        skip_runtime_bounds_check=True,
                    )[1])
                cond = conds[e] if e >= 2 else None
                if e < 2:
                    conds.append(None)
                nbp = 4 if e >= EPC - 2 else NBP
                chunks = []
                jpb = NJ // nbp  # u-blocks per chunk
                csz = KC * NJ * 128 // nbp
                for bp in range(nbp):
                    wc = wp.tile([128, csz], fp8, tag=f"wc{csz}")
                    eng = nc.sync if (e * NBP + bp) % 2 == 0 else nc.scalar
                    eng.dma_start(
                        wc[:],
                        whl[e][:, bp * csz:(bp + 1) * csz],
                        cond=cond,
                    )
                    chunks.append(wc)
                if e == 0:
                    # xt rides right behind e0's first chunks, split
                    # across both rings so their byte totals stay in
                    # phase (an expert's two halves should land
                    # together — skew turns into PE chunk-waits that
                    # cross the HAM re-throttle window).
                    half = EPC * KC * cap2 // 2
                    nc.sync.dma_start(xt_t[:, :half], xt[:, :half])
                    nc.scalar.dma_start(xt_t[:, half:], xt[:, half:])
                ps = pp.tile([128, NJ * cap2], f32)
                for j in range(NJ):
                    wc = chunks[j // jpb]
                    for k in range(KC):
                        nc.tensor.matmul(
                            ps[:, j * cap2:(j + 1) * cap2],
                            lhsT=wc[:, ((j % jpb) * KC + k) * 128:
                                    ((j % jpb) * KC + k) * 128 + 128],
                            rhs=xt_t[:, (e * KC + k) * cap2:
                                     (e * KC + k) * cap2 + cap2],
                            start=(k == 0),
                            stop=(k == KC - 1),
                        )
                # fold the hi/lo x halves: out = hi + lo/32. DVE may
                # read only one PSUM operand per op: scale lo out to
                # SBUF, then add hi.
                tmp = tp.tile([128, NJ * cap], f32, tag="tmp")
                ot = op.tile([128, NJ * cap], f32, tag="ot")
                ps3 = ps[:].rearrange("p (j s) -> p j s", j=NJ)
                tmp3 = tmp[:].rearrange("p (j s) -> p j s", j=NJ)
                ot3 = ot[:].rearrange("p (j s) -> p j s", j=NJ)
                nc.vector.tensor_scalar_mul(tmp3, ps3[:, :, cap:cap2],
                                            1.0 / 32)
                nc.vector.tensor_add(ot3, tmp3, ps3[:, :, 0:cap])
                if e < EPC - 2:
                    # mid-stream outputs ride SWDGE so the HWDGE rings
                    # stay clear for the weight stream
                    nc.gpsimd.dma_start(out[e], ot[:])
                else:
                    held.append((e, ot))
            for (e, ot), eng in zip(held, (nc.sync, nc.scalar)):
                eng.dma_start(out[e], ot[:], cond=conds[e])
    nc.compile()
    return nc


def _get_compiled(cap: int):
    if cap not in _compiled:
        _compiled[cap] = _build_fp8(cap)
    return _compiled[cap]


def _route(content_idx, x, cap):
    """Group examples by expert into padded slots. Returns the packed
    per-expert x [C, cap, D] plus the (expert, slot) of every example."""
    counts = np.bincount(content_idx, minlength=C)
    order = np.argsort(content_idx, kind="stable")
    cs = content_idx[order]
    starts = np.zeros(C, np.int64)
    starts[1:] = np.cumsum(counts)[:-1]
    slot = np.arange(len(content_idx)) - starts[cs]
    xp_ = np.zeros((C, cap, D), np.float32)
    xp_[cs, slot] = x[order]
    return xp_, order, cs, slot


def run(content_idx, x, embeddings, trace=False, trace_cores=None,
        variant="fp8"):
    content_idx = np.asarray(content_idx, np.int32)
    x = np.ascontiguousarray(np.asarray(x, np.float32))
    embeddings = np.ascontiguousarray(np.asarray(embeddings, np.float32))

    counts = np.bincount(content_idx, minlength=C)
    cap = max(12, -(-int(counts.max()) // 4) * 4)

    # Permute experts so unused ones sit in the tail slots of the first
    # cores; their weight DMAs are predicated off on device.
    uq = list(np.where(counts > 0)[0])
    xq = list(np.where(counts == 0)[0])
    new_order = []
    for k in range(NCORES):
        # slots 0-2 are unconditional (never DMA-skipped) so give them
        # used experts; unused experts go in the NEXT slots so their
        # garbage matmuls run at the stream head (hidden under DMA
        # waits), not as a serialized drain after the stream.
        n_head = min(2, len(uq))
        n_un = min(len(xq), EPC - n_head)
        n_tail = EPC - n_head - n_un
        new_order += uq[:n_head] + xq[:n_un] + uq[n_head:n_head + n_tail]
        uq, xq = uq[n_head + n_tail:], xq[n_un:]
    new_order = np.asarray(new_order)
    inv = np.empty(C, np.int64)
    inv[new_order] = np.arange(C)
    content_idx = inv[content_idx].astype(np.int32)
    embeddings = embeddings[new_order]
    counts = counts[new_order]
    emask = (counts > 0).astype(np.int32).reshape(NCORES, EPC)

    xp_, order, cs, slot = _route(content_idx, x, cap)
    cap2 = 2 * cap

    # --- weights: fp8 e3m4 with per-column scales (applied on host) ---
    s_col = np.abs(embeddings).max(axis=1) / 8.0          # [C, U]
    s_col = np.maximum(s_col, 1e-30).astype(np.float32)
    q = (embeddings / s_col[:, None, :]).astype(E3M4)
    # SBUF image: whl[c, p, (j*KC + k)*128 + uu] = q[c, d=k*128+p, u=j*128+uu]
    whl = np.ascontiguousarray(
        q.reshape(C, KC, 128, NJ, 128).transpose(0, 2, 3, 1, 4)
    ).reshape(C, 128, KC * NJ * 128)

    # --- x: fp8 hi/lo, x*g = qxh + qxl/32 ---
    g = np.float32(8.0) / max(float(np.abs(xp_).max()), 1e-30)
    xg = xp_ * g
    xh = xg.astype(E3M4)
    xl = ((xg - xh.astype(np.float32)) * 32.0).astype(E3M4)
    xhl = np.concatenate([xh, xl], axis=1)                # [C, cap2, D]
    # lhsT-side moving image: xt[core, p, (e*KC + k)*cap2 + s]
    xt = np.ascontiguousarray(
        xhl.reshape(NCORES, EPC, cap2, KC, 128).transpose(0, 4, 1, 3, 2)
    ).reshape(NCORES, 128, EPC * KC * cap2)

    nc = _get_compiled(cap)
    in_maps = [
        {"whl": whl[k * EPC:(k + 1) * EPC], "xt": xt[k],
         "emask": np.ascontiguousarray(emask[k][None, :])}
        for k in range(NCORES)
    ]
    res = bass_utils.run_bass_kernel_spmd(
        nc, in_maps, core_ids=list(range(NCORES)),
        trace=trace, trace_cores=trace_cores,
    )
    # out_dev[core, e, p, j*cap + s] = acc[u = j*128 + p, slot s]
    od = np.stack([res.results[k]["out"] for k in range(NCORES)])
    outs = od.reshape(NCORES, EPC, 128, NJ, cap).transpose(0, 1, 4, 3, 2)
    outs = np.ascontiguousarray(outs).reshape(C, cap, U)
    out = np.empty((len(content_idx), U), np.float32)
    out[order] = outs[cs, slot] * s_col[cs] / g
    return out, res


def kernel(content_idx, x, embeddings):
    out, _ = run(content_idx, x, embeddings)
    return out
